# revision 24
# baseline (speedup 1.0000x reference)
"""Trainium2 Bass kernel for a full transformer block (nn_Attention_32873679684330).

Sharding: data-parallel over batch — B=8 batch elements, one per NeuronCore.
Each core runs the full block (LN1 -> QKV -> attention -> out-proj+residual ->
LN2 -> GELU MLP -> residual) on its [1024, 1024] slice, fully on-chip.

Layout: activations are kept feature-major ([features(partitions), tokens(free)])
so every E/MLP contraction feeds the PE array without transposes. Per-token
statistics (LN mean/var, softmax denominators) are computed with ones-vector
matmuls (column sums over partitions) and broadcast back across partitions with
K=1 outer-product matmuls. Attention uses transposed scores sT[j,i] = k_j . q_i
so the softmax denominator comes for free from a ones-column appended to V.

dtypes: residual stream and QKV matmuls in fp32r (fp32 with 12-bit mantissa,
1 PE cycle/row at N=512); attention (q,k,probs,v,o) and FFN in bf16 with fp32
accumulation; all normalization math in fp32.
"""

import sys, os

for _p in ("/root/.axon_site", "/root/.axon_site/_ro/trn_rl_repo",
           "/root/.axon_site/_ro/pypackages"):
    if _p not in sys.path:
        sys.path.append(_p)

import numpy as np
from contextlib import ExitStack

import concourse.bass as bass
import concourse.bacc as bacc
import concourse.mybir as mybir
import concourse.tile as tile
from concourse.bass_utils import run_bass_kernel_spmd

F32 = mybir.dt.float32
F32R = mybir.dt.float32r
BF16 = mybir.dt.bfloat16
NP_BF16 = np.dtype(mybir.dt.np(BF16))
AF = mybir.ActivationFunctionType

B, P, E, H, DH, MLP = 8, 1024, 1024, 16, 64, 4096
SCALE = DH ** -0.5
NCORES = 8
EC = E // 128       # 8 feature chunks
TC = P // 128       # 8 token chunks
TN = P // 512       # 2 token 512-halves
MC = MLP // 128     # 32 mlp chunks


def round_fp32r(x):
    b = np.ascontiguousarray(x, dtype=np.float32).view(np.uint32)
    b = ((b.astype(np.uint64) + 0x800) & 0xFFFFF000).astype(np.uint32)
    return b.view(np.float32)


STAGE_RANK = {"ln1": 0, "qkv": 1, "attn": 2, "x2": 3, "ln2": 4, "full": 9}


def build_program(stage="full"):
    rank = STAGE_RANK[stage]
    nc = bacc.Bacc("TRN2", target_bir_lowering=False, debug=False,
                   num_devices=NCORES)

    xT_d = nc.dram_tensor("xT", [E, P], BF16, kind="ExternalInput").ap()
    wqkv_d = nc.dram_tensor("wqkv", [E, 3 * E], BF16, kind="ExternalInput").ap()
    wo_d = nc.dram_tensor("wo", [E, E], BF16, kind="ExternalInput").ap()
    w1_d = nc.dram_tensor("w1", [E, MLP], BF16, kind="ExternalInput").ap()
    w2_d = nc.dram_tensor("w2", [MLP, E], BF16, kind="ExternalInput").ap()
    bv_row_d = nc.dram_tensor("bv_row", [1, E], F32R, kind="ExternalInput").ap()
    bo_row_d = nc.dram_tensor("bo_row", [1, E], F32R, kind="ExternalInput").ap()
    b2_row_d = nc.dram_tensor("b2_row", [1, E], F32R, kind="ExternalInput").ap()
    bqkv_pm_d = nc.dram_tensor("bqkv_pm", [128, 16], F32, kind="ExternalInput").ap()
    b1_pm_d = nc.dram_tensor("b1_pm", [128, MC], F32, kind="ExternalInput").ap()
    g1_pm_d = nc.dram_tensor("g1_pm", [128, EC], F32, kind="ExternalInput").ap()
    bt1_pm_d = nc.dram_tensor("bt1_pm", [128, EC], F32, kind="ExternalInput").ap()
    g2_pm_d = nc.dram_tensor("g2_pm", [128, EC], F32, kind="ExternalInput").ap()
    bt2_pm_d = nc.dram_tensor("bt2_pm", [128, EC], F32, kind="ExternalInput").ap()
    ones_row_d = nc.dram_tensor("ones_row", [1, 512], F32R, kind="ExternalInput").ap()
    ones_col_d = nc.dram_tensor("ones_col", [128, 1], F32R, kind="ExternalInput").ap()
    ones_col_bf_d = nc.dram_tensor("ones_col_bf", [128, 1], BF16, kind="ExternalInput").ap()
    vones_d = nc.dram_tensor("vones", [128, TC * H], BF16, kind="ExternalInput").ap()

    outT_d = nc.dram_tensor("outT", [E, P], F32, kind="ExternalOutput").ap()
    dbg_d = None
    if stage != "full":
        dbg_d = nc.dram_tensor("dbg", [4 * 1024, P], F32, kind="ExternalOutput").ap()

    with tile.TileContext(nc) as tc, ExitStack() as ctx:
        const = ctx.enter_context(tc.tile_pool(name="const", bufs=1))
        wpool = ctx.enter_context(tc.tile_pool(name="w", bufs=12))
        scr = ctx.enter_context(tc.tile_pool(name="scr", bufs=3))
        rows = ctx.enter_context(tc.tile_pool(name="rows", bufs=2))
        xTp = ctx.enter_context(tc.tile_pool(name="xTp", bufs=1))

        # ---- load xT (bf16, feature-major) ----
        xT = xTp.tile([128, EC, P], BF16, tag="xT", name="xT_sb")
        for c in range(EC):
            nc.sync.dma_start(xT[:, c, :], xT_d[c * 128:(c + 1) * 128, :])

        # ---- constants ----
        def cload(shape, dt, dram, cname):
            t = const.tile(shape, dt, name=cname)
            nc.sync.dma_start(t[:], dram[:])
            return t

        ones_row = cload([1, 512], F32R, ones_row_d, "c_ones_row")
        ones_col = cload([128, 1], F32R, ones_col_d, "c_ones_col")
        ones_col_bf = cload([128, 1], BF16, ones_col_bf_d, "c_ones_col_bf")
        bv_row = cload([1, E], F32R, bv_row_d, "c_bv_row")
        bo_row = cload([1, E], F32R, bo_row_d, "c_bo_row")
        b2_row = cload([1, E], F32R, b2_row_d, "c_b2_row")
        bqkv_pm = cload([128, 16], F32, bqkv_pm_d, "c_bqkv_pm")
        b1_pm = cload([128, MC], F32, b1_pm_d, "c_b1_pm")
        g1_pm = cload([128, EC], F32, g1_pm_d, "c_g1_pm")
        bt1_pm = cload([128, EC], F32, bt1_pm_d, "c_bt1_pm")
        g2_pm = cload([128, EC], F32, g2_pm_d, "c_g2_pm")
        bt2_pm = cload([128, EC], F32, bt2_pm_d, "c_bt2_pm")
        vones = cload([128, TC * H], BF16, vones_d, "c_vones")
        eps_sb = const.tile([1, 1], F32, name="c_eps")
        nc.vector.memset(eps_sb[:], 1e-5)

        def dump_fm(src, row0):
            dpool = tc.alloc_tile_pool(name="dump", bufs=2, side="right")
            for c in range(EC):
                st = dpool.tile([128, P], F32, tag="dump", name=f"dump_{row0}_{c}")
                nc.scalar.activation(st[:], src[:, c, :], AF.Copy)
                nc.sync.dma_start(dbg_d[row0 + c * 128: row0 + (c + 1) * 128, :], st[:])
            dpool.release()

        def layernorm(src_t, onesc, sq_dt, g_pm, b_pm, out_pool, out_dt, nm,
                      ps_pool=None, st_tag="st", st_bufs=2, bc_tag="bc", bc_bufs=2):
            out = out_pool.tile([128, EC, P], out_dt, tag="xn", name=f"{nm}_sb")
            psLN = ps_pool or tc.alloc_tile_pool(name=f"psLN_{nm}", bufs=2,
                                                 space="PSUM")
            mu_rows, rstd_rows = [], []
            for tn in range(TN):
                sl = slice(tn * 512, (tn + 1) * 512)
                mu_ps = psLN.tile([1, 512], F32, tag=st_tag, bufs=st_bufs,
                                  name=f"{nm}_mups{tn}")
                for c in range(EC):
                    nc.tensor.matmul(mu_ps[:], onesc[:], src_t[:, c, sl],
                                     start=(c == 0), stop=(c == EC - 1))
                mu_row = rows.tile([1, 512], F32R, tag="mu", name=f"{nm}_mu{tn}")
                nc.scalar.activation(mu_row[:], mu_ps[:], AF.Copy, scale=1.0 / E)
                sq_ps = psLN.tile([1, 512], F32, tag=st_tag, bufs=st_bufs,
                                  name=f"{nm}_sqps{tn}")
                for c in range(EC):
                    sq = scr.tile([128, 512], sq_dt, tag="sq", name=f"{nm}_sq{tn}_{c}")
                    nc.scalar.activation(sq[:], src_t[:, c, sl], AF.Square)
                    nc.tensor.matmul(sq_ps[:], onesc[:], sq[:],
                                     start=(c == 0), stop=(c == EC - 1))
                msq = rows.tile([1, 512], F32, tag="r", bufs=4, name=f"{nm}_msq{tn}")
                nc.scalar.activation(msq[:], sq_ps[:], AF.Copy, scale=1.0 / E)
                mu2 = rows.tile([1, 512], F32, tag="r", bufs=4, name=f"{nm}_mu2{tn}")
                nc.scalar.activation(mu2[:], mu_row[:], AF.Square)
                var = rows.tile([1, 512], F32, tag="r", bufs=4, name=f"{nm}_var{tn}")
                nc.vector.tensor_sub(var[:], msq[:], mu2[:])
                lv = rows.tile([1, 512], F32, tag="r", bufs=4, name=f"{nm}_lv{tn}")
                nc.scalar.activation(lv[:], var[:], AF.Ln, bias=eps_sb[:])
                rstd_r = rows.tile([1, 512], F32R, tag="mu", name=f"{nm}_rstdr{tn}")
                nc.scalar.activation(rstd_r[:], lv[:], AF.Exp, scale=-0.5)
                mu_rows.append(mu_row)
                rstd_rows.append(rstd_r)
            for tn in range(TN):
                sl = slice(tn * 512, (tn + 1) * 512)
                mu_b = psLN.tile([128, 512], F32, tag=bc_tag, bufs=bc_bufs,
                                 name=f"{nm}_mub{tn}")
                nc.tensor.matmul(mu_b[:], ones_row[:, :128], mu_rows[tn][:],
                                 start=True, stop=True)
                r_b = psLN.tile([128, 512], F32, tag=bc_tag, bufs=bc_bufs,
                                name=f"{nm}_rb{tn}")
                nc.tensor.matmul(r_b[:], ones_row[:, :128], rstd_rows[tn][:],
                                 start=True, stop=True)
                for c in range(EC):
                    d = scr.tile([128, 512], F32, tag="lnd", bufs=4, name=f"{nm}_d{tn}_{c}")
                    nc.vector.tensor_sub(d[:], src_t[:, c, sl], mu_b[:])
                    e = scr.tile([128, 512], F32, tag="lne", bufs=4, name=f"{nm}_e{tn}_{c}")
                    nc.vector.tensor_mul(e[:], d[:], r_b[:])
                    nc.scalar.activation(out[:, c, sl], e[:], AF.Identity,
                                         scale=g_pm[:, c:c + 1], bias=b_pm[:, c:c + 1])
            if ps_pool is None:
                psLN.release()
            return out

        # ======== LN1 ========
        xn1p = tc.alloc_tile_pool(name="xn1p", bufs=1, side="right")
        xnT = layernorm(xT, ones_col_bf, BF16, g1_pm, bt1_pm, xn1p, BF16, "ln1")
        if stage == "ln1":
            dump_fm(xnT, 0)
        if rank < 1:
            xn1p.release()
            return nc

        # ======== QKV ========
        psB = tc.alloc_tile_pool(name="psB", bufs=2, space="PSUM")
        qkvp = tc.alloc_tile_pool(name="qkvp", bufs=1)
        qT = qkvp.tile([128, EC, P], BF16, tag="qT", name="qT_sb")
        kT = qkvp.tile([128, EC, P], BF16, tag="kT", name="kT_sb")
        v_sb = qkvp.tile([128, TC, H, DH + 1], BF16, tag="v", name="v_sb")
        nc.vector.tensor_copy(v_sb[:, :, :, DH],
                              vones[:].rearrange("p (a b) -> p a b", b=H))

        PS_TAGS = [("mm", 2, None), ("o", 2, None), ("sc", 2, [128, 2, 512])]

        def ps_rr(idx, nm):
            tag, bufs, shp = PS_TAGS[idx % 3]
            if shp is None:
                return psB.tile([128, 512], F32, tag=tag, bufs=bufs, name=nm)
            t = psB.tile(shp, F32, tag=tag, bufs=bufs, name=nm)
            return t[:, 0, :]

        for vg in range(2):  # v feature groups of 512
            wts = []
            for ec in range(EC):
                w = wpool.tile([128, 512], BF16, tag="w", name=f"wv_{vg}_{ec}")
                nc.sync.dma_start(w[:], wqkv_d[ec * 128:(ec + 1) * 128,
                                               2 * E + vg * 512: 2 * E + (vg + 1) * 512])
                wts.append(w)
            for tcc in range(TC):
                ps = psB.tile([128, 512], F32, tag="mm", name=f"v_ps{vg}_{tcc}")
                nc.tensor.matmul(ps[:], ones_row[:, :128],
                                 bv_row[:, vg * 512:(vg + 1) * 512],
                                 start=True, stop=False)
                for ec in range(EC):
                    nc.tensor.matmul(ps[:], xnT[:, ec, tcc * 128:(tcc + 1) * 128],
                                     wts[ec][:],
                                     start=False, stop=(ec == EC - 1))
                nc.vector.tensor_copy(
                    v_sb[:, tcc, vg * 8:(vg + 1) * 8, 0:DH],
                    ps[:].rearrange("p (h d) -> p h d", d=DH))
        for fg in (0, 2, 1, 3):  # q (fg 0,1) and k (fg 2,3), interleaved
            wts = []
            for ec in range(EC):
                w = wpool.tile([128, 512], BF16, tag="w", name=f"wqk_{fg}_{ec}")
                nc.sync.dma_start(w[:], wqkv_d[ec * 128:(ec + 1) * 128,
                                               fg * 512:(fg + 1) * 512])
                wts.append(w)
            for fcl in range(4):
                fc = fg * 4 + fcl       # 0..15 over q then k
                dst = qT if fc < EC else kT
                c = fc % EC
                for tn in range(TN):
                    sl = slice(tn * 512, (tn + 1) * 512)
                    ps = psB.tile([128, 512], F32, tag="mm", name=f"qk_ps{fc}_{tn}")
                    for ec in range(EC):
                        nc.tensor.matmul(ps[:], wts[ec][:, fcl * 128:(fcl + 1) * 128],
                                         xnT[:, ec, sl],
                                         start=(ec == 0), stop=(ec == EC - 1))
                    nc.vector.tensor_scalar_add(dst[:, c, sl], ps[:],
                                                bqkv_pm[:, fc:fc + 1])
        xn1p.release()

        if stage == "qkv":
            dpool = tc.alloc_tile_pool(name="dumpq", bufs=2, side="right")
            for c in range(EC):
                for src, r0 in ((qT, 0), (kT, 1024)):
                    st = dpool.tile([128, P], F32, tag="dump", name=f"dq{r0}_{c}")
                    nc.scalar.activation(st[:], src[:, c, :], AF.Copy)
                    nc.sync.dma_start(dbg_d[r0 + c * 128: r0 + (c + 1) * 128, :], st[:])
            for tcc in range(TC):
                st = dpool.tile([128, H * DH], F32, tag="dump", name=f"dv_{tcc}")
                nc.vector.tensor_copy(st[:].rearrange("p (h d) -> p h d", d=DH),
                                      v_sb[:, tcc, :, 0:DH])
                nc.sync.dma_start(dbg_d[2048 + tcc * 128: 2048 + (tcc + 1) * 128, :],
                                  st[:])
            dpool.release()
        if rank < 2:
            qkvp.release()
            return nc

        # ======== attention ========
        attnp = tc.alloc_tile_pool(name="attnp", bufs=1, side="right")
        oT = attnp.tile([128, EC, P], BF16, tag="oT", name="oT_sb")
        for i in range(TN):
            for h in range(H):
                c, pb = h // 2, (h % 2) * DH
                isl = slice(i * 512, (i + 1) * 512)
                aT = attnp.tile([128, TC, 512], BF16, tag="aT", bufs=3,
                                name=f"aT_{h}_{i}")
                for jj in range(TC // 2):
                    sps = psB.tile([128, 2, 512], F32, tag="sc",
                                    name=f"s_ps{h}_{i}_{jj}")
                    for u in range(2):
                        j = 2 * jj + u
                        nc.tensor.matmul(sps[:, u, :],
                                         kT[pb:pb + DH, c, j * 128:(j + 1) * 128],
                                         qT[pb:pb + DH, c, isl],
                                         start=True, stop=True)
                    nc.scalar.activation(aT[:, 2 * jj:2 * jj + 2, :], sps[:],
                                         AF.Exp, scale=SCALE)
                ops = psB.tile([128, 512], F32, tag="o", bufs=2, name=f"o_ps{h}_{i}")
                for j in range(TC):
                    nc.tensor.matmul(ops[0:DH + 1, :], v_sb[:, j, h, :], aT[:, j, :],
                                     start=(j == 0), stop=(j == TC - 1))
                den = rows.tile([1, 512], F32, tag="den", bufs=3,
                                name=f"den_{h}_{i}")
                nc.vector.tensor_copy(den[:], ops[DH:DH + 1, :])
                den_b = scr.tile([DH, 512], F32, tag="denb", name=f"denb_{h}_{i}")
                nc.gpsimd.partition_broadcast(den_b[:], den[:])
                rec = scr.tile([DH, 512], F32, tag="rec", name=f"rec_{h}_{i}")
                nc.vector.reciprocal_approx_fast(rec[:], den_b[:])
                nc.vector.tensor_mul(oT[pb:pb + DH, c, isl], ops[0:DH, :], rec[:])
        qkvp.release()

        if stage == "attn":
            dump_fm(oT, 0)
        if rank < 3:
            psB.release()
            attnp.release()
            return nc

        # ======== out-proj + residual ========
        x2p = tc.alloc_tile_pool(name="x2p", bufs=1)
        x2T = x2p.tile([128, EC, P], F32R, tag="x2T", name="x2T_sb")
        for tn in range(TN):
            sl = slice(tn * 512, (tn + 1) * 512)
            for fg in range(2):
                wts = []
                for ec in range(EC):
                    w = wpool.tile([128, 512], BF16, tag="w", name=f"wo_{tn}_{fg}_{ec}")
                    nc.sync.dma_start(w[:], wo_d[ec * 128:(ec + 1) * 128,
                                                 fg * 512:(fg + 1) * 512])
                    wts.append(w)
                for fcl in range(4):
                    fc = fg * 4 + fcl
                    ps = psB.tile([128, 512], F32, tag="mm", name=f"x2_ps{fc}_{tn}")
                    nc.tensor.matmul(ps[:], bo_row[:, fc * 128:(fc + 1) * 128],
                                     ones_row[:, :512], start=True, stop=False)
                    for ec in range(EC):
                        nc.tensor.matmul(ps[:], wts[ec][:, fcl * 128:(fcl + 1) * 128],
                                         oT[:, ec, sl],
                                         start=False, stop=(ec == EC - 1))
                    nc.vector.tensor_add(x2T[:, fc, sl], ps[:], xT[:, fc, sl])
        attnp.release()
        if stage == "x2":
            dump_fm(x2T, 0)
        if rank < 4:
            psB.release()
            x2p.release()
            return nc

        # ======== LN2 ========
        psB.release()
        psF = tc.alloc_tile_pool(name="psF", bufs=2, space="PSUM")
        xn2p = tc.alloc_tile_pool(name="xn2p", bufs=1, side="right")
        xn2T = layernorm(x2T, ones_col, F32R, g2_pm, bt2_pm, xn2p, BF16, "ln2",
                         ps_pool=psF, st_tag="f1", st_bufs=4,
                         bc_tag="fc", bc_bufs=4)
        if stage == "ln2":
            dump_fm(xn2T, 0)
        if rank < 9:
            xn2p.release()
            psF.release()
            x2p.release()
            return nc

        # ======== FFN (per token half, fused FFN1/FFN2 over m-chunks) ========
        hp = tc.alloc_tile_pool(name="hp", bufs=1, side="right")
        for tn in range(TN):
            sl = slice(tn * 512, (tn + 1) * 512)
            hT = hp.tile([128, MC, 512], BF16, tag="hT", name=f"hT_{tn}")
            for fg in range(2):
                pcs = []
                for fcl in range(4):
                    fc = fg * 4 + fcl
                    pc = psF.tile([128, 512], F32, tag="fc", bufs=4,
                                   name=f"ff_ps{tn}_{fc}")
                    nc.tensor.matmul(pc[:], b2_row[:, fc * 128:(fc + 1) * 128],
                                     ones_row[:, :512], start=True, stop=False)
                    pcs.append(pc)
                for mg in range(8):
                    if fg == 0:
                        w1ts = []
                        for ec in range(EC):
                            w = wpool.tile([128, 512], BF16, tag="w",
                                           name=f"w1_{tn}_{mg}_{ec}")
                            nc.sync.dma_start(w[:], w1_d[ec * 128:(ec + 1) * 128,
                                                         mg * 512:(mg + 1) * 512])
                            w1ts.append(w)
                    for ml in range(4):
                        mc = mg * 4 + ml
                        if fg == 0:
                            hps = psF.tile([128, 512], F32, tag="f1", bufs=4,
                                            name=f"h_ps{tn}_{mc}")
                            for ec in range(EC):
                                nc.tensor.matmul(hps[:],
                                                 w1ts[ec][:, ml * 128:(ml + 1) * 128],
                                                 xn2T[:, ec, sl],
                                                 start=(ec == 0), stop=(ec == EC - 1))
                            nc.scalar.activation(hT[:, mc, :], hps[:], AF.Gelu,
                                                 bias=b1_pm[:, mc:mc + 1])
                        w2t = wpool.tile([128, 512], BF16, tag="w2", bufs=8,
                                         name=f"w2_{tn}_{fg}_{mc}")
                        nc.scalar.dma_start(w2t[:], w2_d[mc * 128:(mc + 1) * 128,
                                                         fg * 512:(fg + 1) * 512])
                        for fcl in range(4):
                            nc.tensor.matmul(pcs[fcl][:],
                                             w2t[:, fcl * 128:(fcl + 1) * 128],
                                             hT[:, mc, :],
                                             start=False, stop=(mc == MC - 1))
                for fcl in range(4):
                    fc = fg * 4 + fcl
                    og = scr.tile([128, 512], F32, tag="og", bufs=3,
                                  name=f"og_{tn}_{fc}")
                    nc.vector.tensor_add(og[:], pcs[fcl][:], x2T[:, fc, sl])
                    nc.sync.dma_start(outT_d[fc * 128:(fc + 1) * 128, sl], og[:])
        hp.release()
        xn2p.release()
        psF.release()
        x2p.release()
    return nc


def prep_inputs(x, ln1_g, ln1_b, wqkv, bqkv, wo, bo, ln2_g, ln2_b, w1, b1, w2, b2):
    """Host-side layout prep: shard x over batch, transpose to feature-major,
    round/cast matmul operands, build partition-major bias/gamma tiles."""
    def pm(vec, nchunks):
        return np.ascontiguousarray(
            np.asarray(vec, dtype=np.float32).reshape(nchunks, 128).T)

    bqkv = np.asarray(bqkv, np.float32)
    shared = dict(
        wqkv=np.asarray(wqkv, np.float32).astype(NP_BF16),
        wo=np.asarray(wo, np.float32).astype(NP_BF16),
        w1=np.asarray(w1, np.float32).astype(NP_BF16),
        w2=np.asarray(w2, np.float32).astype(NP_BF16),
        bv_row=round_fp32r(bqkv[2 * E:].reshape(1, E)),
        bo_row=round_fp32r(np.asarray(bo, np.float32).reshape(1, E)),
        b2_row=round_fp32r(np.asarray(b2, np.float32).reshape(1, E)),
        bqkv_pm=pm(bqkv[:2 * E], 16),
        b1_pm=pm(b1, MC),
        g1_pm=pm(ln1_g, EC),
        bt1_pm=pm(ln1_b, EC),
        g2_pm=pm(ln2_g, EC),
        bt2_pm=pm(ln2_b, EC),
        ones_row=np.ones((1, 512), np.float32),
        ones_col=np.ones((128, 1), np.float32),
        ones_col_bf=np.ones((128, 1), np.float32).astype(NP_BF16),
        vones=np.ones((128, TC * H), np.float32).astype(NP_BF16),
    )
    x = np.asarray(x, np.float32)
    in_maps = []
    for b in range(B):
        m = dict(shared)
        m["xT"] = np.ascontiguousarray(x[b, :, :E].T).astype(NP_BF16)
        in_maps.append(m)
    return in_maps


_CACHE = {}


def run_on_hw(inputs, stage="full", trace=False, **trace_kw):
    key = stage
    if key not in _CACHE:
        nc = build_program(stage)
        nc.compile()
        _CACHE[key] = nc
    nc = _CACHE[key]
    in_maps = prep_inputs(**inputs)
    res = run_bass_kernel_spmd(nc, in_maps, list(range(NCORES)), trace=trace,
                               **trace_kw)
    return res


def kernel(**inputs) -> np.ndarray:
    res = run_on_hw(inputs, stage="full", trace=False)
    out = np.zeros((B, P, E + 1), np.float32)
    for b in range(B):
        out[b, :, :E] = res.results[b]["outT"].T
    return out
